# revision 26
# baseline (speedup 1.0000x reference)
import sys, os
sys.path.insert(0, "/opt/trn_rl_repo")
import numpy as np
import ml_dtypes

from concourse import bass, bacc, tile, mybir
from concourse.bass_utils import run_bass_kernel_spmd

bf16 = mybir.dt.bfloat16
f32 = mybir.dt.float32
i16 = mybir.dt.int16
AF = mybir.ActivationFunctionType
ALU = mybir.AluOpType

NC = 8
H = 128
EPS = 1e-5


def _wrap_idx(a):
    # gather idx layout: token i at [i%16, i//16], replicated to 128 partitions
    n = len(a)
    n16 = (n + 15) // 16
    w = np.zeros((16, n16), np.int16)
    for p in range(16):
        w[p, : len(a[p::16])] = a[p::16]
    return np.tile(w, (8, 1))


def build(cfg):
    """cfg: N, E, L, NPC, NPAD, EC_PAD, wsched (tile->window), flags dict"""
    N, E, L = cfg["N"], cfg["E"], cfg["L"]
    NPC, NPAD = cfg["NPC"], cfg["NPAD"]
    ECP = cfg["EC_PAD"]
    wsched = cfg["wsched"]          # len ET, window index per 128-edge tile
    NW = NPAD // 128                # agg windows per core
    NT = NPAD // 128                # node tiles
    ET = ECP // 128
    ECH = ECP // 512
    assert ET == len(wsched) and ECP % 512 == 0
    TBL = NC * NPAD
    fl = cfg["flags"]
    # first/last flags for scatter psum chaining per window run
    wfirst = [i == 0 or wsched[i] != wsched[i - 1] for i in range(ET)]
    wlast = [i == ET - 1 or wsched[i] != wsched[i + 1] for i in range(ET)]
    last_tile = {}
    for i, wd in enumerate(wsched):
        last_tile[wd] = i

    nc = bacc.Bacc(None, target_bir_lowering=False, num_devices=NC, num_swdge_queues=4)

    P = lambda n, s, d: nc.declare_dram_parameter(n, s, d, isOutput=False)
    xT_d = P("xT", [5, NPAD], bf16)
    eaT_d = P("eaT", [3, ECP], bf16)
    src_d = P("srci", [128, ECP // 16], i16)
    segp_d = P("segp", [2, ET, 128, 128], bf16)   # [0]=seg [e,n], [1]=segT [n,e]
    icnt_d = P("icnt", [128, NT], f32)
    emask_d = P("emask", [128, ET], f32)
    nmask_d = P("nmask", [128, NT], f32)
    ident_d = P("ident", [128, 128], bf16)
    ones1_d = P("ones1", [1, 128], f32)
    onesK_d = P("onesK", [128, 1], f32)
    # weights (bf16, layouts already matmul-ready)
    encNW0_d = P("encNW0", [5, 128], bf16)
    encNW_d = P("encNW", [3, 128, 128], bf16)
    encEW0_d = P("encEW0", [3, 128], bf16)
    encEW_d = P("encEW", [3, 128, 128], bf16)
    eW0_d = P("eW0", [L, 3, 128, 128], bf16)
    eWs0_d = P("eWs0", [L, 128, 128], bf16)
    eWs1_d = P("eWs1", [L, 128, 128], bf16)
    nW0_d = P("nW0", [L, 2, 128, 128], bf16)
    nWs0_d = P("nWs0", [L, 128, 128], bf16)
    nWs1_d = P("nWs1", [L, 128, 128], bf16)
    decW_d = P("decW", [3, 128, 128], bf16)
    decWl_d = P("decWl", [128, 3], bf16)
    # biases feature-major [128, n] f32
    encNb_d = P("encNb", [128, 4], f32)
    encEb_d = P("encEb", [128, 4], f32)
    eb_d = P("eb", [128, 3 * L], f32)
    nb_d = P("nb", [128, 3 * L], f32)
    decb_d = P("decb", [128, 3], f32)
    # optional LN affine broadcast tiles [128,128] (free-dim = feature)
    if fl["eln"]:
        elnw_d = P("elnw", [L, 128, 128], f32)
        elnb_d = P("elnb", [L, 128, 128], f32)
    if fl["nln"]:
        nlnw_d = P("nlnw", [L, 128, 128], f32)
        nlnb_d = P("nlnb", [L, 128, 128], f32)
    if fl["gln"]:
        gNw_d = P("gNw", [128, 128], f32)
        gNb_d = P("gNb", [128, 128], f32)
        gEw_d = P("gEw", [128, 128], f32)
        gEb_d = P("gEb", [128, 128], f32)
    if fl["decbl"]:
        decbl_d = P("decbl", [128, 3], f32)

    out_d = nc.declare_dram_parameter("out", [NPAD, 3], f32, isOutput=True)
    hsh_d = nc.dram_tensor("hsh", [NPAD, 128], bf16)
    htab_d = nc.dram_tensor("htab", [TBL, 128], bf16, addr_space="Shared")
    sti_d = nc.dram_tensor("sti", [4], f32)
    sto_d = nc.dram_tensor("sto", [4], f32, addr_space="Shared")

    RG = [list(range(NC))]

    with tile.TileContext(nc) as tc:
        with (
            tc.tile_pool(name="const", bufs=1) as cp,
            tc.tile_pool(name="big", bufs=1) as bigp,
            tc.tile_pool(name="work", bufs=3) as wp,
            tc.tile_pool(name="stat", bufs=4) as sp,
            tc.tile_pool(name="pMLP", bufs=2, space="PSUM") as pM,
            tc.tile_pool(name="p3", bufs=2, space="PSUM") as p3,
            tc.tile_pool(name="pX", bufs=2, space="PSUM") as pX,
            tc.tile_pool(name="pS", bufs=2, space="PSUM") as pS,
        ):
            # ---- persistent SBUF state ----
            e_rm = bigp.tile([128, ET, 128], f32)      # edge features row-major (master)
            h_own = bigp.tile([128, NT, 128], f32)     # own node features (master)
            h_bf = bigp.tile([128, NT, 128], bf16)     # bf16 row-major shadow
            h_fm = bigp.tile([128, NT, 128], bf16)     # bf16 feature-major shadow
            agg_b = bigp.tile([128, NW, 128], bf16)    # scatter-mean results

            # ---- load constants ----
            def ld(shape, dt, src, tag):
                t = cp.tile(shape, dt, tag=tag)
                nc.sync.dma_start(t[:], src[:])
                return t

            def ldw(src, n, tag, dt=bf16):
                t = cp.tile([128, n, 128], dt, tag=tag)
                nc.sync.dma_start(t[:], src[:].rearrange("n k m -> k n m"))
                return t

            xT = ld([5, NPAD], bf16, xT_d, "xT")
            srci = ld([128, ECP // 16], i16, src_d, "srci")
            icnt = ld([128, NT], f32, icnt_d, "icnt")
            emask = ld([128, ET], f32, emask_d, "emask")
            nmask = ld([128, NT], f32, nmask_d, "nmask")
            ident = ld([128, 128], bf16, ident_d, "ident")
            ones1 = ld([1, 128], f32, ones1_d, "ones1")
            onesK = ld([128, 1], f32, onesK_d, "onesK")
            encNW0 = ld([5, 128], bf16, encNW0_d, "encNW0")
            encNW = ldw(encNW_d, 3, "encNW")
            encEW0 = ld([3, 128], bf16, encEW0_d, "encEW0")
            encEW = ldw(encEW_d, 3, "encEW")
            eW0 = cp.tile([128, L * 3, 128], bf16, tag="eW0")
            nc.sync.dma_start(eW0[:], eW0_d[:].rearrange("l n k m -> k (l n) m"))
            eWs0 = ldw(eWs0_d, L, "eWs0")
            eWs1 = ldw(eWs1_d, L, "eWs1")
            nW0 = cp.tile([128, L * 2, 128], bf16, tag="nW0")
            nc.sync.dma_start(nW0[:], nW0_d[:].rearrange("l n k m -> k (l n) m"))
            nWs0 = ldw(nWs0_d, L, "nWs0")
            nWs1 = ldw(nWs1_d, L, "nWs1")
            decW = ldw(decW_d, 3, "decW")
            decWl = ld([128, 3], bf16, decWl_d, "decWl")
            encNb = ld([128, 4], f32, encNb_d, "encNb")
            encEb = ld([128, 4], f32, encEb_d, "encEb")
            eb = ld([128, 3 * L], f32, eb_d, "eb")
            nb = ld([128, 3 * L], f32, nb_d, "nb")
            decb = ld([128, 3], f32, decb_d, "decb")
            if fl["eln"]:
                elnw = ldw(elnw_d, L, "elnw", f32)
                elnb = ldw(elnb_d, L, "elnb", f32)
            if fl["nln"]:
                nlnw = ldw(nlnw_d, L, "nlnw", f32)
                nlnb = ldw(nlnb_d, L, "nlnb", f32)
            if fl["gln"]:
                gNw = ld([128, 128], f32, gNw_d, "gNw")
                gNb = ld([128, 128], f32, gNb_d, "gNb")
                gEw = ld([128, 128], f32, gEw_d, "gEw")
                gEb = ld([128, 128], f32, gEb_d, "gEb")
            if fl["decbl"]:
                decbl = ld([128, 3], f32, decbl_d, "decbl")

            # graph-LN stats accumulators [128,4]
            hsum4 = sp.tile([128, 4], f32, tag="hsum4")
            hsq4 = sp.tile([128, 4], f32, tag="hsq4")
            esum4 = sp.tile([128, 4], f32, tag="esum4")
            esq4 = sp.tile([128, 4], f32, tag="esq4")
            for t_ in (hsum4, hsq4, esum4, esq4):
                nc.vector.memset(t_[:], 0.0)
            epsA = sp.tile([128, 1], f32, tag="epsA")
            nc.vector.memset(epsA[:], EPS)

            def mlp_head(rhs_list, w1_list, b1, w2, b2, w3=None, b3=None, width=512):
                """Feature-major MLP head: a=relu(sum_i w1_i.T@rhs_i + b1);
                a2=relu(w2.T@a + b2); optional third layer. bf16 [128,width]."""
                ps = pM.tile([128, width], f32, tag="pM")
                for i, (w, r) in enumerate(zip(w1_list, rhs_list)):
                    nc.tensor.matmul(ps[:], w, r, start=(i == 0),
                                     stop=(i == len(w1_list) - 1))
                a = wp.tile([128, width], bf16, tag="a1")
                nc.scalar.activation(a[:], ps[:], AF.Relu, bias=b1)
                ps2 = pM.tile([128, width], f32, tag="pM")
                nc.tensor.matmul(ps2[:], w2, a[:], start=True, stop=True)
                a2 = wp.tile([128, width], bf16, tag="a2")
                nc.scalar.activation(a2[:], ps2[:], AF.Relu, bias=b2)
                if w3 is None:
                    return a2
                ps3 = pM.tile([128, width], f32, tag="pM")
                nc.tensor.matmul(ps3[:], w3, a2[:], start=True, stop=True)
                a3 = wp.tile([128, width], bf16, tag="a2")
                nc.scalar.activation(a3[:], ps3[:], AF.Relu, bias=b3)
                return a3

            def rowln_factors(ps3, ntile):
                """Per-row LN over ps3 [128, ntile, 128] psum.
                Returns (rs, nmr) [128, :ntile]."""
                s1 = sp.tile([128, 4], f32, tag="s1")
                nc.vector.tensor_reduce(s1[:, :ntile], ps3[:, :ntile, :],
                                        mybir.AxisListType.X, ALU.add)
                scr = wp.tile([128, 4, 128], f32, tag="lnscr")
                nc.scalar.activation(scr[:, :ntile, :], ps3[:, :ntile, :], AF.Square)
                s2 = sp.tile([128, 4], f32, tag="s2")
                nc.vector.tensor_reduce(s2[:, :ntile], scr[:, :ntile, :],
                                        mybir.AxisListType.X, ALU.add)
                mu = sp.tile([128, 4], f32, tag="mu")
                nc.vector.tensor_scalar(mu[:, :ntile], s1[:, :ntile], 1.0 / 128, None, ALU.mult)
                mu2 = sp.tile([128, 4], f32, tag="mu2")
                nc.scalar.activation(mu2[:, :ntile], mu[:, :ntile], AF.Square)
                var = sp.tile([128, 4], f32, tag="var")
                nc.vector.scalar_tensor_tensor(var[:, :ntile], s2[:, :ntile], 1.0 / 128,
                                               mu2[:, :ntile], ALU.mult, ALU.subtract)
                sd = sp.tile([128, 4], f32, tag="sd")
                nc.scalar.activation(sd[:, :ntile], var[:, :ntile], AF.Sqrt, bias=epsA[:])
                rs = sp.tile([128, 4], f32, tag="rs")
                nc.vector.reciprocal(rs[:, :ntile], sd[:, :ntile])
                nmr = sp.tile([128, 4], f32, tag="nmr")
                nc.vector.scalar_tensor_tensor(nmr[:, :ntile], mu[:, :ntile], -1.0,
                                               rs[:, :ntile], ALU.mult, ALU.mult)
                return rs, nmr

            def transpose4(src_tiles, ntile, out_dt=bf16, tag="pX"):
                """PE-transpose ntile [128,128] bf16 tiles into one psum tile."""
                psT = pX.tile([128, 4, 128], out_dt, tag=tag)
                for j in range(ntile):
                    nc.tensor.transpose(psT[:, j, :], src_tiles[j], ident[:])
                return psT

            # ================= NODE ENCODER =================
            NCH = (NPAD + 511) // 512
            for c in range(NCH):
                c0 = c * 512
                w = min(512, NPAD - c0)
                nt4 = w // 128
                a3 = mlp_head(
                    [xT[:, c0 : c0 + w]], [encNW0[:]],
                    encNb[:, 0:1], encNW[:, 0, :], encNb[:, 1:2], encNW[:, 1, :], encNb[:, 2:3],
                    width=w,
                )
                ps3 = p3.tile([128, 4, 128], f32, tag="p3")
                for j in range(nt4):
                    nc.tensor.matmul(ps3[:, j, :], a3[:, j * 128 : (j + 1) * 128],
                                     encNW[:, 2, :], start=True, stop=True)
                nc.scalar.activation(h_own[:, 4 * c : 4 * c + nt4, :], ps3[:, :nt4, :], AF.Copy)
                s1 = sp.tile([128, 4], f32, tag="s1")
                nc.vector.tensor_reduce(s1[:, :nt4], ps3[:, :nt4, :],
                                        mybir.AxisListType.X, ALU.add)
                scr = wp.tile([128, 4, 128], f32, tag="lnscr")
                nc.scalar.activation(scr[:, :nt4, :], ps3[:, :nt4, :], AF.Square)
                s2 = sp.tile([128, 4], f32, tag="s2")
                nc.vector.tensor_reduce(s2[:, :nt4], scr[:, :nt4, :],
                                        mybir.AxisListType.X, ALU.add)
                m1 = sp.tile([128, 4], f32, tag="m1")
                nc.vector.tensor_tensor(m1[:, :nt4], s1[:, :nt4], nmask[:, 4 * c : 4 * c + nt4], ALU.mult)
                nc.vector.tensor_tensor(hsum4[:, :nt4], hsum4[:, :nt4], m1[:, :nt4], ALU.add)
                nc.vector.tensor_tensor(m1[:, :nt4], s2[:, :nt4], nmask[:, 4 * c : 4 * c + nt4], ALU.mult)
                nc.vector.tensor_tensor(hsq4[:, :nt4], hsq4[:, :nt4], m1[:, :nt4], ALU.add)

            # ================= EDGE ENCODER =================
            for c in range(ECH):
                c0 = c * 512
                eat = wp.tile([3, 512], bf16, tag="eat")
                nc.sync.dma_start(eat[:], eaT_d[:, c0 : c0 + 512])
                a3 = mlp_head(
                    [eat[:]], [encEW0[:]],
                    encEb[:, 0:1], encEW[:, 0, :], encEb[:, 1:2], encEW[:, 1, :], encEb[:, 2:3],
                )
                ps3 = p3.tile([128, 4, 128], f32, tag="p3")
                for j in range(4):
                    nc.tensor.matmul(ps3[:, j, :], a3[:, j * 128 : (j + 1) * 128],
                                     encEW[:, 2, :], start=True, stop=True)
                nc.scalar.activation(e_rm[:, 4 * c : 4 * c + 4, :], ps3[:], AF.Copy)
                s1 = sp.tile([128, 4], f32, tag="s1")
                nc.vector.tensor_reduce(s1[:], ps3[:], mybir.AxisListType.X, ALU.add)
                scr = wp.tile([128, 4, 128], f32, tag="lnscr")
                nc.scalar.activation(scr[:], ps3[:], AF.Square)
                s2 = sp.tile([128, 4], f32, tag="s2")
                nc.vector.tensor_reduce(s2[:], scr[:], mybir.AxisListType.X, ALU.add)
                m1 = sp.tile([128, 4], f32, tag="m1")
                nc.vector.tensor_tensor(m1[:], s1[:], emask[:, 4 * c : 4 * c + 4], ALU.mult)
                nc.vector.tensor_tensor(esum4[:], esum4[:], m1[:], ALU.add)
                nc.vector.tensor_tensor(m1[:], s2[:], emask[:, 4 * c : 4 * c + 4], ALU.mult)
                nc.vector.tensor_tensor(esq4[:], esq4[:], m1[:], ALU.add)

            # ============ GLOBAL GRAPH-LN STATS ============
            st4 = sp.tile([128, 4], f32, tag="st4")
            for j, t_ in enumerate((hsum4, hsq4, esum4, esq4)):
                nc.vector.tensor_reduce(st4[:, j : j + 1], t_[:], mybir.AxisListType.X, ALU.add)
            psst = p3.tile([128, 4, 128], f32, tag="p3")
            nc.tensor.matmul(psst[:4, 0, :1], st4[:], onesK[:], start=True, stop=True)
            stv = sp.tile([4, 1], f32, tag="stv")
            nc.scalar.activation(stv[:], psst[:4, 0, :1], AF.Copy)
            nc.sync.dma_start(sti_d[:], stv[:, 0:1])
            nc.gpsimd.collective_compute(
                "AllReduce", ALU.add, replica_groups=RG, ins=[sti_d[:]], outs=[sto_d[:]]
            )
            st14 = sp.tile([1, 4], f32, tag="st14")
            nc.sync.dma_start(st14[:], sto_d[:])
            psb = p3.tile([128, 4, 128], f32, tag="p3")
            nc.tensor.matmul(psb[:, 0, :4], ones1[:], st14[:], start=True, stop=True)
            stb = sp.tile([128, 4], f32, tag="stb")
            nc.scalar.activation(stb[:], psb[:, 0, :4], AF.Copy)

            def graph_ln_factors(sumc, sqc, count):
                mu = sp.tile([128, 1], f32, tag="gmu")
                nc.vector.tensor_scalar(mu[:], sumc, 1.0 / count, None, ALU.mult)
                e2 = sp.tile([128, 1], f32, tag="ge2")
                nc.vector.tensor_scalar(e2[:], sqc, 1.0 / count, None, ALU.mult)
                mu2 = sp.tile([128, 1], f32, tag="gmu2")
                nc.scalar.activation(mu2[:], mu[:], AF.Square)
                var = sp.tile([128, 1], f32, tag="gvar")
                nc.vector.tensor_tensor(var[:], e2[:], mu2[:], ALU.subtract)
                sd = sp.tile([128, 1], f32, tag="gsd")
                nc.scalar.activation(sd[:], var[:], AF.Sqrt)
                nc.vector.tensor_scalar(sd[:], sd[:], EPS, None, ALU.add)
                r = sp.tile([128, 1], f32, tag="gr")
                nc.vector.reciprocal(r[:], sd[:])
                nmr = sp.tile([128, 1], f32, tag="gnmr")
                nc.vector.tensor_scalar(nmr[:], mu[:], r[:], -1.0, ALU.mult, ALU.mult)
                return r, nmr

            rh, nmrh = graph_ln_factors(stb[:, 0:1], stb[:, 1:2], float(N) * H)
            re, nmre = graph_ln_factors(stb[:, 2:3], stb[:, 3:4], float(E) * H)

            KL = int(os.environ.get("KLAYERS", L))
            lazy = (KL == L) and (not fl["gln"]) and L >= 1 and not os.environ.get("NOLAZY")

            def update_h_shadows(c, nt4):
                """cast h chunk -> h_bf, DMA to hsh, PE-transpose into h_fm."""
                c0 = c * 512
                w = nt4 * 128
                nc.scalar.activation(h_bf[:, 4 * c : 4 * c + nt4, :],
                                     h_own[:, 4 * c : 4 * c + nt4, :], AF.Copy)
                nc.sync.dma_start(
                    hsh_d[c0 : c0 + w].rearrange("(t p) f -> p t f", p=128),
                    h_bf[:, 4 * c : 4 * c + nt4, :])
                psT = transpose4([h_bf[:, 4 * c + j, :] for j in range(nt4)], nt4)
                nc.scalar.activation(h_fm[:, 4 * c : 4 * c + nt4, :],
                                     psT[:, :nt4, :], AF.Copy)

            if lazy:
                # raw h shadows + table allgather overlap the stats allreduce;
                # graph-LN folds into layer-0 weights/biases below
                for c in range(NCH):
                    update_h_shadows(c, min(4, NT - 4 * c))
                nc.gpsimd.collective_compute(
                    "AllGather", ALU.bypass, replica_groups=RG, ins=[hsh_d[:]], outs=[htab_d[:]]
                )
                onesKb = sp.tile([128, 1], bf16, tag="onesKb")
                nc.scalar.activation(onesKb[:], onesK[:], AF.Copy)
                eA1 = sp.tile([128, 128], bf16, tag="eA1")
                nc.vector.tensor_scalar(eA1[:], eW0[:, 0, :], rh[:], None, ALU.mult)
                eB1 = sp.tile([128, 128], bf16, tag="eB1")
                nc.vector.tensor_scalar(eB1[:], eW0[:, 1, :], rh[:], None, ALU.mult)
                eC1 = sp.tile([128, 128], bf16, tag="eC1")
                nc.vector.tensor_scalar(eC1[:], eW0[:, 2, :], re[:], None, ALU.mult)
                nA1 = sp.tile([128, 128], bf16, tag="nA1")
                nc.vector.tensor_scalar(nA1[:], nW0[:, 0, :], rh[:], None, ALU.mult)
                psC = p3.tile([128, 4, 128], f32, tag="p3")
                for j, wmat in enumerate((eW0[:, 0, :], eW0[:, 1, :], eW0[:, 2, :], nW0[:, 0, :])):
                    nc.tensor.matmul(psC[:, j, 0:1], wmat, onesKb[:], start=True, stop=True)
                cs4 = sp.tile([128, 4, 1], f32, tag="cs4")
                nc.scalar.activation(cs4[:], psC[:, :, 0:1], AF.Copy)
                tA = sp.tile([128, 1], f32, tag="tA")
                nc.vector.tensor_tensor(tA[:], cs4[:, 0, :], cs4[:, 1, :], ALU.add)
                nc.vector.tensor_scalar(tA[:], tA[:], nmrh[:], None, ALU.mult)
                tC = sp.tile([128, 1], f32, tag="tC")
                nc.vector.tensor_scalar(tC[:], cs4[:, 2, :], nmre[:], None, ALU.mult)
                eb1f = sp.tile([128, 1], f32, tag="eb1f")
                nc.vector.tensor_tensor(eb1f[:], eb[:, 0:1], tA[:], ALU.add)
                nc.vector.tensor_tensor(eb1f[:], eb1f[:], tC[:], ALU.add)
                nb1f = sp.tile([128, 1], f32, tag="nb1f")
                nc.vector.tensor_scalar(nb1f[:], cs4[:, 3, :], nmrh[:], None, ALU.mult)
                nc.vector.tensor_tensor(nb1f[:], nb[:, 0:1], nb1f[:], ALU.add)
            else:
                # eager apply of graph-LN to h master + build shadows
                for c in range(NCH):
                    nt4 = min(4, NT - 4 * c)
                    nc.vector.tensor_scalar(h_own[:, 4 * c : 4 * c + nt4, :],
                                            h_own[:, 4 * c : 4 * c + nt4, :],
                                            rh[:], nmrh[:], ALU.mult, ALU.add)
                    if fl["gln"]:
                        for j in range(nt4):
                            t = 4 * c + j
                            nc.vector.tensor_tensor(h_own[:, t, :], h_own[:, t, :], gNw[:], ALU.mult)
                            nc.vector.tensor_tensor(h_own[:, t, :], h_own[:, t, :], gNb[:], ALU.add)
                    update_h_shadows(c, nt4)
                for c in range(ECH):
                    nc.vector.tensor_scalar(e_rm[:, 4 * c : 4 * c + 4, :],
                                            e_rm[:, 4 * c : 4 * c + 4, :],
                                            re[:], nmre[:], ALU.mult, ALU.add)
                    if fl["gln"]:
                        for j in range(4):
                            t = 4 * c + j
                            nc.vector.tensor_tensor(e_rm[:, t, :], e_rm[:, t, :], gEw[:], ALU.mult)
                            nc.vector.tensor_tensor(e_rm[:, t, :], e_rm[:, t, :], gEb[:], ALU.add)
                nc.gpsimd.collective_compute(
                    "AllGather", ALU.bypass, replica_groups=RG, ins=[hsh_d[:]], outs=[htab_d[:]]
                )

            # ================= MP LAYERS =================
            # node chunk c is emitted right after the edge chunk that closes
            # its last dst window (wsched sorted -> later edge chunks only
            # read h_bf windows beyond it, so the interleave is hazard-free)
            NCH_n = (NT + 3) // 4
            ecreq = {}
            for cn in range(NCH_n):
                ntn = min(4, NT - 4 * cn)
                req = max(last_tile[4 * cn + j] for j in range(ntn)) // 4
                ecreq.setdefault(req, []).append(cn)

            for l in range(int(os.environ.get("KLAYERS", L))):
                def node_chunk(c, l=l):
                    nt4 = min(4, NT - 4 * c)
                    w = nt4 * 128
                    if lazy and l == 0:
                        wn, bn = [nA1[:], nW0[:, 1, :]], nb1f[:]
                    else:
                        wn = [nW0[:, 2 * l, :], nW0[:, 2 * l + 1, :]]
                        bn = nb[:, 3 * l : 3 * l + 1]
                    a2 = mlp_head(
                        [h_fm[:, 4 * c : 4 * c + nt4, :], agg_b[:, 4 * c : 4 * c + nt4, :]],
                        wn, bn, nWs0[:, l, :], nb[:, 3 * l + 1 : 3 * l + 2],
                        width=w,
                    )
                    ps3 = p3.tile([128, 4, 128], f32, tag="p3")
                    for j in range(nt4):
                        nc.tensor.matmul(ps3[:, j, :], a2[:, j * 128 : (j + 1) * 128],
                                         nWs1[:, l, :], start=True, stop=True)
                    rs, nmr = rowln_factors(ps3, nt4)
                    tmpf = wp.tile([128, 4, 128], f32, tag="tmpf")
                    for j in range(nt4):
                        nc.vector.tensor_scalar(tmpf[:, j, :], ps3[:, j, :],
                                                rs[:, j : j + 1], nmr[:, j : j + 1],
                                                ALU.mult, ALU.add)
                    if fl["nln"]:
                        for j in range(nt4):
                            nc.vector.tensor_tensor(tmpf[:, j, :], tmpf[:, j, :], nlnw[:, l, :], ALU.mult)
                            nc.vector.tensor_tensor(tmpf[:, j, :], tmpf[:, j, :], nlnb[:, l, :], ALU.add)
                    if lazy and l == 0:
                        nc.vector.tensor_scalar(h_own[:, 4 * c : 4 * c + nt4, :],
                                                h_own[:, 4 * c : 4 * c + nt4, :],
                                                rh[:], nmrh[:], ALU.mult, ALU.add)
                    nc.vector.tensor_tensor(h_own[:, 4 * c : 4 * c + nt4, :],
                                            h_own[:, 4 * c : 4 * c + nt4, :],
                                            tmpf[:, :nt4, :], ALU.add)
                    update_h_shadows(c, nt4)

                # -------- edge phase (node chunks interleaved) --------
                psW = None
                for c in range(ECH):
                    segld = wp.tile([128, 2, 4, 128], bf16, tag="segld", bufs=6)
                    for k in range(2):
                        nc.sync.dma_start(segld[:, k, :, :],
                                          segp_d[k, 4 * c : 4 * c + 4].rearrange(
                                              "t p f -> p t f"))
                    hsrc = wp.tile([128, 1, 512], bf16, tag="hsrc", bufs=10)
                    nc.gpsimd.dma_gather(hsrc[:], htab_d[:], srci[:, c * 32 : c * 32 + 32],
                                         512, 512, 128, transpose=True, queue_num=c % 4)
                    # dst-side h expansion: one-hot segT selects own-node rows.
                    # segld[:, 4+j, :] = segT tile j; window runs share lhsT so
                    # emit one slab matmul per run.
                    psE = pX.tile([128, 4, 128], f32, tag="pX")
                    j = 0
                    while j < 4:
                        wdx = wsched[4 * c + j]
                        j1 = j
                        while j1 + 1 < 4 and wsched[4 * c + j1 + 1] == wdx:
                            j1 += 1
                        nc.tensor.matmul(psE[:, j : j1 + 1, :], h_bf[:, wdx, :],
                                         segld[:, 1, j : j1 + 1, :],
                                         start=True, stop=True)
                        j = j1 + 1
                    hdst = wp.tile([128, 4, 128], bf16, tag="hdst")
                    nc.scalar.activation(hdst[:], psE[:], AF.Copy)
                    # e chunk row->feature-major via PE transpose
                    ebr = wp.tile([128, 4, 128], bf16, tag="ebr")
                    nc.scalar.activation(ebr[:], e_rm[:, 4 * c : 4 * c + 4, :], AF.Copy)
                    psT = transpose4([ebr[:, j, :] for j in range(4)], 4)
                    efm = wp.tile([128, 4, 128], bf16, tag="efm")
                    nc.scalar.activation(efm[:], psT[:], AF.Copy)
                    # hsrc (gather-dependent) last: hides the collective+gather
                    if lazy and l == 0:
                        wl, bl = [eA1[:], eC1[:], eB1[:]], eb1f[:]
                    else:
                        wl = [eW0[:, 3 * l, :], eW0[:, 3 * l + 2, :], eW0[:, 3 * l + 1, :]]
                        bl = eb[:, 3 * l : 3 * l + 1]
                    a2 = mlp_head(
                        [hdst[:], efm[:], hsrc[:, 0, :]], wl,
                        bl, eWs0[:, l, :], eb[:, 3 * l + 1 : 3 * l + 2],
                    )
                    ps3 = p3.tile([128, 4, 128], f32, tag="p3")
                    for j in range(4):
                        nc.tensor.matmul(ps3[:, j, :], a2[:, j * 128 : (j + 1) * 128],
                                         eWs1[:, l, :], start=True, stop=True)
                    rs, nmr = rowln_factors(ps3, 4)
                    tmpf = wp.tile([128, 4, 128], f32, tag="tmpf")
                    for j in range(4):
                        if j % 2 == 0:
                            nc.scalar.activation(tmpf[:, j, :], ps3[:, j, :], AF.Identity,
                                                 bias=nmr[:, j : j + 1], scale=rs[:, j : j + 1])
                        else:
                            nc.vector.tensor_scalar(tmpf[:, j, :], ps3[:, j, :],
                                                    rs[:, j : j + 1], nmr[:, j : j + 1],
                                                    ALU.mult, ALU.add)
                    if fl["eln"]:
                        for j in range(4):
                            nc.vector.tensor_tensor(tmpf[:, j, :], tmpf[:, j, :], elnw[:, l, :], ALU.mult)
                            nc.vector.tensor_tensor(tmpf[:, j, :], tmpf[:, j, :], elnb[:, l, :], ALU.add)
                    if lazy and l == 0:
                        nc.vector.tensor_scalar(e_rm[:, 4 * c : 4 * c + 4, :],
                                                e_rm[:, 4 * c : 4 * c + 4, :],
                                                re[:], nmre[:], ALU.mult, ALU.add)
                    nc.vector.tensor_tensor(e_rm[:, 4 * c : 4 * c + 4, :],
                                            e_rm[:, 4 * c : 4 * c + 4, :], tmpf[:], ALU.add)
                    tmpb = wp.tile([128, 4, 128], bf16, tag="tmpb")
                    nc.scalar.activation(tmpb[:], tmpf[:], AF.Copy)
                    # scatter into window-chained psum (FM out: tmpb is lhsT);
                    # the mean 1/cnt is pre-scaled into seg on the host
                    for j in range(4):
                        t = 4 * c + j
                        wdx = wsched[t]
                        if wfirst[t]:
                            psW = pS.tile([128, 128], f32, tag="pS")
                        nc.tensor.matmul(psW[:], tmpb[:, j, :], segld[:, 0, j, :],
                                         start=wfirst[t], stop=wlast[t])
                        if wlast[t]:
                            nc.vector.tensor_copy(agg_b[:, wdx, :], psW[:])
                    for cn in ecreq.get(c, []):
                        node_chunk(cn)

                if l + 1 < int(os.environ.get("KLAYERS", L)):
                    nc.gpsimd.collective_compute(
                        "AllGather", ALU.bypass, replica_groups=RG,
                        ins=[hsh_d[:]], outs=[htab_d[:]]
                    )

            # ================= DECODER =================
            for c in range(NCH):
                c0 = c * 512
                nt4 = min(4, NT - 4 * c)
                w = nt4 * 128
                a3 = mlp_head(
                    [h_fm[:, 4 * c : 4 * c + nt4, :]], [decW[:, 0, :]],
                    decb[:, 0:1], decW[:, 1, :], decb[:, 1:2], decW[:, 2, :], decb[:, 2:3],
                    width=w,
                )
                psd = p3.tile([128, 4, 128], f32, tag="p3")
                for j in range(nt4):
                    nc.tensor.matmul(psd[:, j, 0:3], a3[:, j * 128 : (j + 1) * 128],
                                     decWl[:], start=True, stop=True)
                ot = wp.tile([128, 4, 3], f32, tag="ot")
                nc.scalar.activation(ot[:, :nt4, :], psd[:, :nt4, 0:3], AF.Copy)
                if fl["decbl"]:
                    for j in range(nt4):
                        nc.vector.tensor_tensor(ot[:, j, :], ot[:, j, :], decbl[:], ALU.add)
                nc.sync.dma_start(
                    out_d[c0 : c0 + w].rearrange("(t p) f -> p t f", p=128),
                    ot[:, :nt4, :])

    nc.compile()
    return nc


def _prep(inputs, cfg):
    """Host-side sharding/index prep. Returns in_maps list."""
    N, E, L = cfg["N"], cfg["E"], cfg["L"]
    NPC, NPAD, ECP = cfg["NPC"], cfg["NPAD"], cfg["EC_PAD"]
    wsched = cfg["wsched"]
    ET = ECP // 128
    NW = NPAD // 128
    f = lambda k: np.asarray(inputs[k], np.float32)
    b = lambda a: np.ascontiguousarray(a).astype(ml_dtypes.bfloat16)

    ei = np.asarray(inputs["edge_index"])
    src_g, dst_g = ei[0].astype(np.int64), ei[1].astype(np.int64)
    ea = f("edge_attr")
    x = f("x")
    cnt = np.bincount(dst_g, minlength=N).astype(np.float32)
    icnt_full = 1.0 / np.maximum(cnt, 1.0)

    tblrow = lambda g: (g // NPC) * NPAD + (g % NPC)

    order = np.argsort(dst_g, kind="stable")
    in_maps = []
    shared = None
    for c in range(NC):
        lo, hi = c * NPC, (c + 1) * NPC
        sel = order[(dst_g[order] >= lo) & (dst_g[order] < hi)]
        dl = dst_g[sel] - lo           # local dst, ascending
        win = dl // 128
        srcv = np.zeros(ECP, np.int64)
        eav = np.zeros((ECP, 3), np.float32)
        msk = np.zeros(ECP, np.float32)
        seg = np.zeros((ET, 128, 128), np.float32)
        seg1 = np.zeros((ET, 128, 128), np.float32)
        pos = {}
        for t, wd in enumerate(wsched):
            pos.setdefault(wd, []).append(t)
        for wd in range(NW):
            idxs = np.where(win == wd)[0]
            tiles = pos.get(wd, [])
            assert len(idxs) <= len(tiles) * 128, (c, wd, len(idxs), len(tiles))
            for k, i in enumerate(idxs):
                t = tiles[k // 128]
                r = k % 128
                g = t * 128 + r
                e_ = sel[i]
                srcv[g] = src_g[e_]
                eav[g] = ea[e_]
                msk[g] = 1.0
                seg[t, r, dl[i] - 128 * wd] = icnt_full[dst_g[e_]]
                seg1[t, r, dl[i] - 128 * wd] = 1.0
        segp = np.zeros((2, ET, 128, 128), np.float32)
        segp[0] = seg
        segp[1] = seg1.transpose(0, 2, 1)
        emask = msk.reshape(ET, 128).T.copy()  # [128, ET]
        nmask = np.zeros((128, NW), np.float32)
        icnt_c = np.ones((128, NW), np.float32)
        for t in range(NW):
            for p in range(128):
                r = t * 128 + p
                if r < NPC:
                    nmask[p, t] = 1.0
                    icnt_c[p, t] = icnt_full[lo + r]
        xT = np.zeros((5, NPAD), np.float32)
        xT[:, :NPC] = x[lo:hi].T
        eaT = eav.T.copy()

        m = {
            "xT": b(xT), "eaT": b(eaT),
            "srci": _wrap_idx(tblrow(srcv).astype(np.int16)),
            "segp": b(segp), "icnt": icnt_c, "emask": emask, "nmask": nmask,
        }
        if shared is None:
            shared = {
                "ident": b(np.eye(128)),
                "ones1": np.ones((1, 128), np.float32),
                "onesK": np.ones((128, 1), np.float32),
                "encNW0": b(f("encN_W0")), "encNW": b(f("encN_Ws")),
                "encEW0": b(f("encE_W0")), "encEW": b(f("encE_Ws")),
                "eW0": b(f("eW0").reshape(L, 3, 128, 128)),
                "eWs0": b(f("eWs")[:, 0]), "eWs1": b(f("eWs")[:, 1]),
                "nW0": b(f("nW0").reshape(L, 2, 128, 128)),
                "nWs0": b(f("nWs")[:, 0]), "nWs1": b(f("nWs")[:, 1]),
                "decW": b(np.stack([f("dec_W0"), f("dec_Ws")[0], f("dec_Ws")[1]])),
                "decWl": b(f("dec_Wl")),
                "encNb": f("encN_bs").T.copy(), "encEb": f("encE_bs").T.copy(),
                "eb": f("ebs").reshape(L * 3, 128).T.copy(),
                "nb": f("nbs").reshape(L * 3, 128).T.copy(),
                "decb": f("dec_bs").T.copy(),
            }
            flg = cfg["flags"]
            if flg["eln"]:
                shared["elnw"] = np.tile(f("elnw")[:, None, :], (1, 128, 1))
                shared["elnb"] = np.tile(f("elnb")[:, None, :], (1, 128, 1))
            if flg["nln"]:
                shared["nlnw"] = np.tile(f("nlnw")[:, None, :], (1, 128, 1))
                shared["nlnb"] = np.tile(f("nlnb")[:, None, :], (1, 128, 1))
            if flg["gln"]:
                shared["gNw"] = np.tile(f("encN_lnw")[None, :], (128, 1))
                shared["gNb"] = np.tile(f("encN_lnb")[None, :], (128, 1))
                shared["gEw"] = np.tile(f("encE_lnw")[None, :], (128, 1))
                shared["gEb"] = np.tile(f("encE_lnb")[None, :], (128, 1))
            if flg["decbl"]:
                shared["decbl"] = np.tile(f("dec_bl")[None, :], (128, 1))
        m.update(shared)
        in_maps.append(m)
    return in_maps


def make_cfg(inputs):
    N = np.asarray(inputs["x"]).shape[0]
    E = np.asarray(inputs["edge_index"]).shape[1]
    L = np.asarray(inputs["eW0"]).shape[0]
    NPC = N // NC
    NPAD = ((NPC + 127) // 128) * 128
    NW = NPAD // 128
    ei = np.asarray(inputs["edge_index"])
    dst = ei[1].astype(np.int64)
    tw = []
    for wd in range(NW):
        mx = 1
        for c in range(NC):
            lo = c * NPC
            nwin = int(((dst >= lo + wd * 128) & (dst < min(lo + (wd + 1) * 128, lo + NPC))).sum())
            mx = max(mx, (nwin + 127) // 128)
        tw.append(mx)
    wsched = []
    for wd in range(NW):
        wsched += [wd] * tw[wd]
    while (len(wsched) * 128) % 512:
        wsched.append(NW - 1)
    flags = {
        "eln": bool(np.any(np.asarray(inputs["elnw"]) != 1) or np.any(np.asarray(inputs["elnb"]) != 0)),
        "nln": bool(np.any(np.asarray(inputs["nlnw"]) != 1) or np.any(np.asarray(inputs["nlnb"]) != 0)),
        "gln": bool(
            np.any(np.asarray(inputs["encN_lnw"]) != 1) or np.any(np.asarray(inputs["encN_lnb"]) != 0)
            or np.any(np.asarray(inputs["encE_lnw"]) != 1) or np.any(np.asarray(inputs["encE_lnb"]) != 0)
        ),
        "decbl": bool(np.any(np.asarray(inputs["dec_bl"]) != 0)),
    }
    return {
        "N": N, "E": E, "L": L, "NPC": NPC, "NPAD": NPAD,
        "EC_PAD": len(wsched) * 128, "wsched": wsched, "flags": flags,
    }


_CACHE = {}


def kernel(**inputs) -> np.ndarray:
    cfg = make_cfg(inputs)
    key = (cfg["N"], cfg["E"], cfg["L"], cfg["EC_PAD"], tuple(sorted(cfg["flags"].items())))
    if key not in _CACHE:
        _CACHE[key] = build(cfg)
    nc = _CACHE[key]
    in_maps = _prep(inputs, cfg)
    res = run_bass_kernel_spmd(nc, in_maps, list(range(NC))).results
    NPC = cfg["NPC"]
    out = np.concatenate([res[c]["out"][:NPC] for c in range(NC)], axis=0)
    return out.astype(np.float32)


# revision 27
# speedup vs baseline: 1.0606x; 1.0606x over previous
import sys, os
sys.path.insert(0, "/opt/trn_rl_repo")
import numpy as np
import ml_dtypes

from concourse import bass, bacc, tile, mybir
from concourse.bass_utils import run_bass_kernel_spmd

bf16 = mybir.dt.bfloat16
f32 = mybir.dt.float32
i16 = mybir.dt.int16
AF = mybir.ActivationFunctionType
ALU = mybir.AluOpType

NC = 8
H = 128
EPS = 1e-5


def _wrap_idx(a):
    # gather idx layout: token i at [i%16, i//16], replicated to 128 partitions
    n = len(a)
    n16 = (n + 15) // 16
    w = np.zeros((16, n16), np.int16)
    for p in range(16):
        w[p, : len(a[p::16])] = a[p::16]
    return np.tile(w, (8, 1))


def build(cfg):
    """cfg: N, E, L, NPC, NPAD, EC_PAD, wsched (tile->window), flags dict"""
    N, E, L = cfg["N"], cfg["E"], cfg["L"]
    NPC, NPAD = cfg["NPC"], cfg["NPAD"]
    ECP = cfg["EC_PAD"]
    wsched = cfg["wsched"]          # len ET, window index per 128-edge tile
    NW = NPAD // 128                # agg windows per core
    NT = NPAD // 128                # node tiles
    ET = ECP // 128
    ECH = ECP // 512
    assert ET == len(wsched) and ECP % 512 == 0
    TBL = NC * NPAD
    fl = cfg["flags"]
    # first/last flags for scatter psum chaining per window run
    wfirst = [i == 0 or wsched[i] != wsched[i - 1] for i in range(ET)]
    wlast = [i == ET - 1 or wsched[i] != wsched[i + 1] for i in range(ET)]
    last_tile = {}
    for i, wd in enumerate(wsched):
        last_tile[wd] = i

    nc = bacc.Bacc(None, target_bir_lowering=False, num_devices=NC, num_swdge_queues=4)

    P = lambda n, s, d: nc.declare_dram_parameter(n, s, d, isOutput=False)
    xT_d = P("xT", [5, NPAD], bf16)
    eaT_d = P("eaT", [3, ECP], bf16)
    src_d = P("srci", [128, ECP // 16], i16)
    segp_d = P("segp", [2, ET, 128, 128], bf16)   # [0]=seg [e,n], [1]=segT [n,e]
    icnt_d = P("icnt", [128, NT], f32)
    emask_d = P("emask", [128, ET], f32)
    nmask_d = P("nmask", [128, NT], f32)
    ident_d = P("ident", [128, 128], bf16)
    ones1_d = P("ones1", [1, 128], f32)
    onesK_d = P("onesK", [128, 1], f32)
    # weights (bf16, layouts already matmul-ready)
    encNW0_d = P("encNW0", [5, 128], bf16)
    encNW_d = P("encNW", [3, 128, 128], bf16)
    encEW0_d = P("encEW0", [3, 128], bf16)
    encEW_d = P("encEW", [3, 128, 128], bf16)
    eW0_d = P("eW0", [L, 3, 128, 128], bf16)
    eWs0_d = P("eWs0", [L, 128, 128], bf16)
    eWs1_d = P("eWs1", [L, 128, 128], bf16)
    nW0_d = P("nW0", [L, 2, 128, 128], bf16)
    nWs0_d = P("nWs0", [L, 128, 128], bf16)
    nWs1_d = P("nWs1", [L, 128, 128], bf16)
    decW_d = P("decW", [3, 128, 128], bf16)
    decWl_d = P("decWl", [128, 3], bf16)
    # biases feature-major [128, n] f32
    encNb_d = P("encNb", [128, 4], f32)
    encEb_d = P("encEb", [128, 4], f32)
    eb_d = P("eb", [128, 3 * L], f32)
    nb_d = P("nb", [128, 3 * L], f32)
    decb_d = P("decb", [128, 3], f32)
    # optional LN affine broadcast tiles [128,128] (free-dim = feature)
    if fl["eln"]:
        elnw_d = P("elnw", [L, 128, 128], f32)
        elnb_d = P("elnb", [L, 128, 128], f32)
    if fl["nln"]:
        nlnw_d = P("nlnw", [L, 128, 128], f32)
        nlnb_d = P("nlnb", [L, 128, 128], f32)
    if fl["gln"]:
        gNw_d = P("gNw", [128, 128], f32)
        gNb_d = P("gNb", [128, 128], f32)
        gEw_d = P("gEw", [128, 128], f32)
        gEb_d = P("gEb", [128, 128], f32)
    if fl["decbl"]:
        decbl_d = P("decbl", [128, 3], f32)

    out_d = nc.declare_dram_parameter("out", [NPAD, 3], f32, isOutput=True)
    hsh_d = nc.dram_tensor("hsh", [NPAD, 128], bf16)
    htab_d = nc.dram_tensor("htab", [TBL, 128], bf16, addr_space="Shared")
    sti_d = nc.dram_tensor("sti", [4], f32)
    sto_d = nc.dram_tensor("sto", [4], f32, addr_space="Shared")

    RG = [list(range(NC))]

    with tile.TileContext(nc) as tc:
        with (
            tc.tile_pool(name="const", bufs=1) as cp,
            tc.tile_pool(name="big", bufs=1) as bigp,
            tc.tile_pool(name="work", bufs=3) as wp,
            tc.tile_pool(name="stat", bufs=4) as sp,
            tc.tile_pool(name="pMLP", bufs=2, space="PSUM") as pM,
            tc.tile_pool(name="p3", bufs=2, space="PSUM") as p3,
            tc.tile_pool(name="pX", bufs=2, space="PSUM") as pX,
            tc.tile_pool(name="pS", bufs=2, space="PSUM") as pS,
        ):
            # ---- persistent SBUF state ----
            e_rm = bigp.tile([128, ET, 128], f32)      # edge features row-major (master)
            h_own = bigp.tile([128, NT, 128], f32)     # own node features (master)
            h_bf = bigp.tile([128, NT, 128], bf16)     # bf16 row-major shadow
            h_fm = bigp.tile([128, NT, 128], bf16)     # bf16 feature-major shadow
            agg_b = bigp.tile([128, NW, 128], bf16)    # scatter-mean results

            # ---- load constants ----
            def ld(shape, dt, src, tag):
                t = cp.tile(shape, dt, tag=tag)
                nc.sync.dma_start(t[:], src[:])
                return t

            def ldw(src, n, tag, dt=bf16):
                t = cp.tile([128, n, 128], dt, tag=tag)
                nc.sync.dma_start(t[:], src[:].rearrange("n k m -> k n m"))
                return t

            xT = ld([5, NPAD], bf16, xT_d, "xT")
            srci = ld([128, ECP // 16], i16, src_d, "srci")
            icnt = ld([128, NT], f32, icnt_d, "icnt")
            emask = ld([128, ET], f32, emask_d, "emask")
            nmask = ld([128, NT], f32, nmask_d, "nmask")
            ident = ld([128, 128], bf16, ident_d, "ident")
            ones1 = ld([1, 128], f32, ones1_d, "ones1")
            onesK = ld([128, 1], f32, onesK_d, "onesK")
            encNW0 = ld([5, 128], bf16, encNW0_d, "encNW0")
            encNW = ldw(encNW_d, 3, "encNW")
            encEW0 = ld([3, 128], bf16, encEW0_d, "encEW0")
            encEW = ldw(encEW_d, 3, "encEW")
            eW0 = cp.tile([128, L * 3, 128], bf16, tag="eW0")
            nc.sync.dma_start(eW0[:], eW0_d[:].rearrange("l n k m -> k (l n) m"))
            eWs0 = ldw(eWs0_d, L, "eWs0")
            eWs1 = ldw(eWs1_d, L, "eWs1")
            nW0 = cp.tile([128, L * 2, 128], bf16, tag="nW0")
            nc.sync.dma_start(nW0[:], nW0_d[:].rearrange("l n k m -> k (l n) m"))
            nWs0 = ldw(nWs0_d, L, "nWs0")
            nWs1 = ldw(nWs1_d, L, "nWs1")
            decW = ldw(decW_d, 3, "decW")
            decWl = ld([128, 3], bf16, decWl_d, "decWl")
            encNb = ld([128, 4], f32, encNb_d, "encNb")
            encEb = ld([128, 4], f32, encEb_d, "encEb")
            eb = ld([128, 3 * L], f32, eb_d, "eb")
            nb = ld([128, 3 * L], f32, nb_d, "nb")
            decb = ld([128, 3], f32, decb_d, "decb")
            if fl["eln"]:
                elnw = ldw(elnw_d, L, "elnw", f32)
                elnb = ldw(elnb_d, L, "elnb", f32)
            if fl["nln"]:
                nlnw = ldw(nlnw_d, L, "nlnw", f32)
                nlnb = ldw(nlnb_d, L, "nlnb", f32)
            if fl["gln"]:
                gNw = ld([128, 128], f32, gNw_d, "gNw")
                gNb = ld([128, 128], f32, gNb_d, "gNb")
                gEw = ld([128, 128], f32, gEw_d, "gEw")
                gEb = ld([128, 128], f32, gEb_d, "gEb")
            if fl["decbl"]:
                decbl = ld([128, 3], f32, decbl_d, "decbl")

            # graph-LN stats accumulators [128,4]
            hsum4 = sp.tile([128, 4], f32, tag="hsum4")
            hsq4 = sp.tile([128, 4], f32, tag="hsq4")
            esum4 = sp.tile([128, 4], f32, tag="esum4")
            esq4 = sp.tile([128, 4], f32, tag="esq4")
            for t_ in (hsum4, hsq4, esum4, esq4):
                nc.vector.memset(t_[:], 0.0)
            epsA = sp.tile([128, 1], f32, tag="epsA")
            nc.vector.memset(epsA[:], EPS)

            def mlp_head(rhs_list, w1_list, b1, w2, b2, w3=None, b3=None, width=512):
                """Feature-major MLP head: a=relu(sum_i w1_i.T@rhs_i + b1);
                a2=relu(w2.T@a + b2); optional third layer. bf16 [128,width]."""
                ps = pM.tile([128, width], f32, tag="pM")
                for i, (w, r) in enumerate(zip(w1_list, rhs_list)):
                    nc.tensor.matmul(ps[:], w, r, start=(i == 0),
                                     stop=(i == len(w1_list) - 1))
                a = wp.tile([128, width], bf16, tag="a1")
                nc.scalar.activation(a[:], ps[:], AF.Relu, bias=b1)
                ps2 = pM.tile([128, width], f32, tag="pM")
                nc.tensor.matmul(ps2[:], w2, a[:], start=True, stop=True)
                a2 = wp.tile([128, width], bf16, tag="a2")
                nc.scalar.activation(a2[:], ps2[:], AF.Relu, bias=b2)
                if w3 is None:
                    return a2
                ps3 = pM.tile([128, width], f32, tag="pM")
                nc.tensor.matmul(ps3[:], w3, a2[:], start=True, stop=True)
                a3 = wp.tile([128, width], bf16, tag="a2")
                nc.scalar.activation(a3[:], ps3[:], AF.Relu, bias=b3)
                return a3

            def rowln_factors(ps3, ntile):
                """Per-row LN over ps3 [128, ntile, 128] psum.
                Returns (rs, nmr) [128, :ntile]."""
                s1 = sp.tile([128, 4], f32, tag="s1")
                nc.vector.tensor_reduce(s1[:, :ntile], ps3[:, :ntile, :],
                                        mybir.AxisListType.X, ALU.add)
                scr = wp.tile([128, 4, 128], f32, tag="lnscr")
                nc.scalar.activation(scr[:, :ntile, :], ps3[:, :ntile, :], AF.Square)
                s2 = sp.tile([128, 4], f32, tag="s2")
                nc.vector.tensor_reduce(s2[:, :ntile], scr[:, :ntile, :],
                                        mybir.AxisListType.X, ALU.add)
                mu = sp.tile([128, 4], f32, tag="mu")
                nc.vector.tensor_scalar(mu[:, :ntile], s1[:, :ntile], 1.0 / 128, None, ALU.mult)
                mu2 = sp.tile([128, 4], f32, tag="mu2")
                nc.scalar.activation(mu2[:, :ntile], mu[:, :ntile], AF.Square)
                var = sp.tile([128, 4], f32, tag="var")
                nc.vector.scalar_tensor_tensor(var[:, :ntile], s2[:, :ntile], 1.0 / 128,
                                               mu2[:, :ntile], ALU.mult, ALU.subtract)
                sd = sp.tile([128, 4], f32, tag="sd")
                nc.scalar.activation(sd[:, :ntile], var[:, :ntile], AF.Sqrt, bias=epsA[:])
                rs = sp.tile([128, 4], f32, tag="rs")
                nc.vector.reciprocal(rs[:, :ntile], sd[:, :ntile])
                nmr = sp.tile([128, 4], f32, tag="nmr")
                nc.vector.scalar_tensor_tensor(nmr[:, :ntile], mu[:, :ntile], -1.0,
                                               rs[:, :ntile], ALU.mult, ALU.mult)
                return rs, nmr

            def transpose4(src_tiles, ntile, out_dt=bf16, tag="pX"):
                """PE-transpose ntile [128,128] bf16 tiles into one psum tile."""
                psT = pX.tile([128, 4, 128], out_dt, tag=tag)
                for j in range(ntile):
                    nc.tensor.transpose(psT[:, j, :], src_tiles[j], ident[:])
                return psT

            # ================= NODE ENCODER =================
            NCH = (NPAD + 511) // 512
            for c in range(NCH):
                c0 = c * 512
                w = min(512, NPAD - c0)
                nt4 = w // 128
                a3 = mlp_head(
                    [xT[:, c0 : c0 + w]], [encNW0[:]],
                    encNb[:, 0:1], encNW[:, 0, :], encNb[:, 1:2], encNW[:, 1, :], encNb[:, 2:3],
                    width=w,
                )
                ps3 = p3.tile([128, 4, 128], f32, tag="p3")
                for j in range(nt4):
                    nc.tensor.matmul(ps3[:, j, :], a3[:, j * 128 : (j + 1) * 128],
                                     encNW[:, 2, :], start=True, stop=True)
                nc.scalar.activation(h_own[:, 4 * c : 4 * c + nt4, :], ps3[:, :nt4, :], AF.Copy)
                s1 = sp.tile([128, 4], f32, tag="s1")
                nc.vector.tensor_reduce(s1[:, :nt4], ps3[:, :nt4, :],
                                        mybir.AxisListType.X, ALU.add)
                scr = wp.tile([128, 4, 128], f32, tag="lnscr")
                nc.scalar.activation(scr[:, :nt4, :], ps3[:, :nt4, :], AF.Square)
                s2 = sp.tile([128, 4], f32, tag="s2")
                nc.vector.tensor_reduce(s2[:, :nt4], scr[:, :nt4, :],
                                        mybir.AxisListType.X, ALU.add)
                m1 = sp.tile([128, 4], f32, tag="m1")
                nc.vector.tensor_tensor(m1[:, :nt4], s1[:, :nt4], nmask[:, 4 * c : 4 * c + nt4], ALU.mult)
                nc.vector.tensor_tensor(hsum4[:, :nt4], hsum4[:, :nt4], m1[:, :nt4], ALU.add)
                nc.vector.tensor_tensor(m1[:, :nt4], s2[:, :nt4], nmask[:, 4 * c : 4 * c + nt4], ALU.mult)
                nc.vector.tensor_tensor(hsq4[:, :nt4], hsq4[:, :nt4], m1[:, :nt4], ALU.add)

            # ================= EDGE ENCODER =================
            for c in range(ECH):
                c0 = c * 512
                eat = wp.tile([3, 512], bf16, tag="eat")
                nc.sync.dma_start(eat[:], eaT_d[:, c0 : c0 + 512])
                a3 = mlp_head(
                    [eat[:]], [encEW0[:]],
                    encEb[:, 0:1], encEW[:, 0, :], encEb[:, 1:2], encEW[:, 1, :], encEb[:, 2:3],
                )
                ps3 = p3.tile([128, 4, 128], f32, tag="p3")
                for j in range(4):
                    nc.tensor.matmul(ps3[:, j, :], a3[:, j * 128 : (j + 1) * 128],
                                     encEW[:, 2, :], start=True, stop=True)
                nc.scalar.activation(e_rm[:, 4 * c : 4 * c + 4, :], ps3[:], AF.Copy)
                s1 = sp.tile([128, 4], f32, tag="s1")
                nc.vector.tensor_reduce(s1[:], ps3[:], mybir.AxisListType.X, ALU.add)
                scr = wp.tile([128, 4, 128], f32, tag="lnscr")
                nc.scalar.activation(scr[:], ps3[:], AF.Square)
                s2 = sp.tile([128, 4], f32, tag="s2")
                nc.vector.tensor_reduce(s2[:], scr[:], mybir.AxisListType.X, ALU.add)
                m1 = sp.tile([128, 4], f32, tag="m1")
                nc.vector.tensor_tensor(m1[:], s1[:], emask[:, 4 * c : 4 * c + 4], ALU.mult)
                nc.vector.tensor_tensor(esum4[:], esum4[:], m1[:], ALU.add)
                nc.vector.tensor_tensor(m1[:], s2[:], emask[:, 4 * c : 4 * c + 4], ALU.mult)
                nc.vector.tensor_tensor(esq4[:], esq4[:], m1[:], ALU.add)

            # ============ GLOBAL GRAPH-LN STATS ============
            st4 = sp.tile([128, 4], f32, tag="st4")
            for j, t_ in enumerate((hsum4, hsq4, esum4, esq4)):
                nc.vector.tensor_reduce(st4[:, j : j + 1], t_[:], mybir.AxisListType.X, ALU.add)
            psst = p3.tile([128, 4, 128], f32, tag="p3")
            nc.tensor.matmul(psst[:4, 0, :1], st4[:], onesK[:], start=True, stop=True)
            stv = sp.tile([4, 1], f32, tag="stv")
            nc.scalar.activation(stv[:], psst[:4, 0, :1], AF.Copy)
            nc.sync.dma_start(sti_d[:], stv[:, 0:1])
            nc.gpsimd.collective_compute(
                "AllReduce", ALU.add, replica_groups=RG, ins=[sti_d[:]], outs=[sto_d[:]]
            )
            st14 = sp.tile([1, 4], f32, tag="st14")
            nc.sync.dma_start(st14[:], sto_d[:])
            psb = p3.tile([128, 4, 128], f32, tag="p3")
            nc.tensor.matmul(psb[:, 0, :4], ones1[:], st14[:], start=True, stop=True)
            stb = sp.tile([128, 4], f32, tag="stb")
            nc.scalar.activation(stb[:], psb[:, 0, :4], AF.Copy)

            def graph_ln_factors(sumc, sqc, count):
                mu = sp.tile([128, 1], f32, tag="gmu")
                nc.vector.tensor_scalar(mu[:], sumc, 1.0 / count, None, ALU.mult)
                e2 = sp.tile([128, 1], f32, tag="ge2")
                nc.vector.tensor_scalar(e2[:], sqc, 1.0 / count, None, ALU.mult)
                mu2 = sp.tile([128, 1], f32, tag="gmu2")
                nc.scalar.activation(mu2[:], mu[:], AF.Square)
                var = sp.tile([128, 1], f32, tag="gvar")
                nc.vector.tensor_tensor(var[:], e2[:], mu2[:], ALU.subtract)
                sd = sp.tile([128, 1], f32, tag="gsd")
                nc.scalar.activation(sd[:], var[:], AF.Sqrt)
                nc.vector.tensor_scalar(sd[:], sd[:], EPS, None, ALU.add)
                r = sp.tile([128, 1], f32, tag="gr")
                nc.vector.reciprocal(r[:], sd[:])
                nmr = sp.tile([128, 1], f32, tag="gnmr")
                nc.vector.tensor_scalar(nmr[:], mu[:], r[:], -1.0, ALU.mult, ALU.mult)
                return r, nmr

            rh, nmrh = graph_ln_factors(stb[:, 0:1], stb[:, 1:2], float(N) * H)
            re, nmre = graph_ln_factors(stb[:, 2:3], stb[:, 3:4], float(E) * H)

            KL = int(os.environ.get("KLAYERS", L))
            lazy = (KL == L) and (not fl["gln"]) and L >= 1 and not os.environ.get("NOLAZY")

            def update_h_shadows(c, nt4):
                """cast h chunk -> h_bf, DMA to hsh, PE-transpose into h_fm."""
                c0 = c * 512
                w = nt4 * 128
                nc.scalar.activation(h_bf[:, 4 * c : 4 * c + nt4, :],
                                     h_own[:, 4 * c : 4 * c + nt4, :], AF.Copy)
                nc.sync.dma_start(
                    hsh_d[c0 : c0 + w].rearrange("(t p) f -> p t f", p=128),
                    h_bf[:, 4 * c : 4 * c + nt4, :])
                psT = transpose4([h_bf[:, 4 * c + j, :] for j in range(nt4)], nt4)
                nc.scalar.activation(h_fm[:, 4 * c : 4 * c + nt4, :],
                                     psT[:, :nt4, :], AF.Copy)

            if lazy:
                # raw h shadows + table allgather overlap the stats allreduce;
                # graph-LN folds into layer-0 weights/biases below
                for c in range(NCH):
                    update_h_shadows(c, min(4, NT - 4 * c))
                nc.gpsimd.collective_compute(
                    "AllGather", ALU.bypass, replica_groups=RG, ins=[hsh_d[:]], outs=[htab_d[:]]
                )
                onesKb = sp.tile([128, 1], bf16, tag="onesKb")
                nc.scalar.activation(onesKb[:], onesK[:], AF.Copy)
                eA1 = sp.tile([128, 128], bf16, tag="eA1")
                nc.vector.tensor_scalar(eA1[:], eW0[:, 0, :], rh[:], None, ALU.mult)
                eB1 = sp.tile([128, 128], bf16, tag="eB1")
                nc.vector.tensor_scalar(eB1[:], eW0[:, 1, :], rh[:], None, ALU.mult)
                eC1 = sp.tile([128, 128], bf16, tag="eC1")
                nc.vector.tensor_scalar(eC1[:], eW0[:, 2, :], re[:], None, ALU.mult)
                nA1 = sp.tile([128, 128], bf16, tag="nA1")
                nc.vector.tensor_scalar(nA1[:], nW0[:, 0, :], rh[:], None, ALU.mult)
                psC = p3.tile([128, 4, 128], f32, tag="p3")
                for j, wmat in enumerate((eW0[:, 0, :], eW0[:, 1, :], eW0[:, 2, :], nW0[:, 0, :])):
                    nc.tensor.matmul(psC[:, j, 0:1], wmat, onesKb[:], start=True, stop=True)
                cs4 = sp.tile([128, 4, 1], f32, tag="cs4")
                nc.scalar.activation(cs4[:], psC[:, :, 0:1], AF.Copy)
                tA = sp.tile([128, 1], f32, tag="tA")
                nc.vector.tensor_tensor(tA[:], cs4[:, 0, :], cs4[:, 1, :], ALU.add)
                nc.vector.tensor_scalar(tA[:], tA[:], nmrh[:], None, ALU.mult)
                tC = sp.tile([128, 1], f32, tag="tC")
                nc.vector.tensor_scalar(tC[:], cs4[:, 2, :], nmre[:], None, ALU.mult)
                eb1f = sp.tile([128, 1], f32, tag="eb1f")
                nc.vector.tensor_tensor(eb1f[:], eb[:, 0:1], tA[:], ALU.add)
                nc.vector.tensor_tensor(eb1f[:], eb1f[:], tC[:], ALU.add)
                nb1f = sp.tile([128, 1], f32, tag="nb1f")
                nc.vector.tensor_scalar(nb1f[:], cs4[:, 3, :], nmrh[:], None, ALU.mult)
                nc.vector.tensor_tensor(nb1f[:], nb[:, 0:1], nb1f[:], ALU.add)
            else:
                # eager apply of graph-LN to h master + build shadows
                for c in range(NCH):
                    nt4 = min(4, NT - 4 * c)
                    nc.vector.tensor_scalar(h_own[:, 4 * c : 4 * c + nt4, :],
                                            h_own[:, 4 * c : 4 * c + nt4, :],
                                            rh[:], nmrh[:], ALU.mult, ALU.add)
                    if fl["gln"]:
                        for j in range(nt4):
                            t = 4 * c + j
                            nc.vector.tensor_tensor(h_own[:, t, :], h_own[:, t, :], gNw[:], ALU.mult)
                            nc.vector.tensor_tensor(h_own[:, t, :], h_own[:, t, :], gNb[:], ALU.add)
                    update_h_shadows(c, nt4)
                for c in range(ECH):
                    nc.vector.tensor_scalar(e_rm[:, 4 * c : 4 * c + 4, :],
                                            e_rm[:, 4 * c : 4 * c + 4, :],
                                            re[:], nmre[:], ALU.mult, ALU.add)
                    if fl["gln"]:
                        for j in range(4):
                            t = 4 * c + j
                            nc.vector.tensor_tensor(e_rm[:, t, :], e_rm[:, t, :], gEw[:], ALU.mult)
                            nc.vector.tensor_tensor(e_rm[:, t, :], e_rm[:, t, :], gEb[:], ALU.add)
                nc.gpsimd.collective_compute(
                    "AllGather", ALU.bypass, replica_groups=RG, ins=[hsh_d[:]], outs=[htab_d[:]]
                )

            # ================= MP LAYERS =================
            # node chunk c is emitted right after the edge chunk that closes
            # its last dst window (wsched sorted -> later edge chunks only
            # read h_bf windows beyond it, so the interleave is hazard-free)
            NCH_n = (NT + 3) // 4
            ecreq = {}
            for cn in range(NCH_n):
                ntn = min(4, NT - 4 * cn)
                req = max(last_tile[4 * cn + j] for j in range(ntn)) // 4
                ecreq.setdefault(req, []).append(cn)

            for l in range(int(os.environ.get("KLAYERS", L))):
                def node_chunk(c, l=l):
                    nt4 = min(4, NT - 4 * c)
                    w = nt4 * 128
                    if lazy and l == 0:
                        wn, bn = [nA1[:], nW0[:, 1, :]], nb1f[:]
                    else:
                        wn = [nW0[:, 2 * l, :], nW0[:, 2 * l + 1, :]]
                        bn = nb[:, 3 * l : 3 * l + 1]
                    a2 = mlp_head(
                        [h_fm[:, 4 * c : 4 * c + nt4, :], agg_b[:, 4 * c : 4 * c + nt4, :]],
                        wn, bn, nWs0[:, l, :], nb[:, 3 * l + 1 : 3 * l + 2],
                        width=w,
                    )
                    ps3 = p3.tile([128, 4, 128], f32, tag="p3")
                    for j in range(nt4):
                        nc.tensor.matmul(ps3[:, j, :], a2[:, j * 128 : (j + 1) * 128],
                                         nWs1[:, l, :], start=True, stop=True)
                    rs, nmr = rowln_factors(ps3, nt4)
                    tmpf = wp.tile([128, 4, 128], f32, tag="tmpf")
                    for j in range(nt4):
                        nc.vector.tensor_scalar(tmpf[:, j, :], ps3[:, j, :],
                                                rs[:, j : j + 1], nmr[:, j : j + 1],
                                                ALU.mult, ALU.add)
                    if fl["nln"]:
                        for j in range(nt4):
                            nc.vector.tensor_tensor(tmpf[:, j, :], tmpf[:, j, :], nlnw[:, l, :], ALU.mult)
                            nc.vector.tensor_tensor(tmpf[:, j, :], tmpf[:, j, :], nlnb[:, l, :], ALU.add)
                    if lazy and l == 0:
                        nc.vector.tensor_scalar(h_own[:, 4 * c : 4 * c + nt4, :],
                                                h_own[:, 4 * c : 4 * c + nt4, :],
                                                rh[:], nmrh[:], ALU.mult, ALU.add)
                    nc.vector.tensor_tensor(h_own[:, 4 * c : 4 * c + nt4, :],
                                            h_own[:, 4 * c : 4 * c + nt4, :],
                                            tmpf[:, :nt4, :], ALU.add)
                    update_h_shadows(c, nt4)

                # -------- edge phase (node chunks interleaved) --------
                psW = None
                for c in range(ECH):
                    segld = wp.tile([128, 2, 4, 128], bf16, tag="segld", bufs=6)
                    for k in range(2):
                        nc.sync.dma_start(segld[:, k, :, :],
                                          segp_d[k, 4 * c : 4 * c + 4].rearrange(
                                              "t p f -> p t f"))
                    hsrc = wp.tile([128, 1, 512], bf16, tag="hsrc", bufs=10)
                    nc.gpsimd.dma_gather(hsrc[:], htab_d[:], srci[:, c * 32 : c * 32 + 32],
                                         512, 512, 128, transpose=True, queue_num=c % 4)
                    # dst-side h expansion: one-hot segT selects own-node rows.
                    # segld[:, 4+j, :] = segT tile j; window runs share lhsT so
                    # emit one slab matmul per run.
                    psE = pX.tile([128, 4, 128], f32, tag="pX")
                    j = 0
                    while j < 4:
                        wdx = wsched[4 * c + j]
                        j1 = j
                        while j1 + 1 < 4 and wsched[4 * c + j1 + 1] == wdx:
                            j1 += 1
                        nc.tensor.matmul(psE[:, j : j1 + 1, :], h_bf[:, wdx, :],
                                         segld[:, 1, j : j1 + 1, :],
                                         start=True, stop=True)
                        j = j1 + 1
                    hdst = wp.tile([128, 4, 128], bf16, tag="hdst")
                    nc.scalar.activation(hdst[:], psE[:], AF.Copy)
                    # e chunk row->feature-major via PE transpose
                    ebr = wp.tile([128, 4, 128], bf16, tag="ebr")
                    nc.scalar.activation(ebr[:], e_rm[:, 4 * c : 4 * c + 4, :], AF.Copy)
                    psT = transpose4([ebr[:, j, :] for j in range(4)], 4)
                    efm = wp.tile([128, 4, 128], bf16, tag="efm")
                    nc.scalar.activation(efm[:], psT[:], AF.Copy)
                    # hsrc (gather-dependent) last: hides the collective+gather
                    if lazy and l == 0:
                        wl, bl = [eA1[:], eC1[:], eB1[:]], eb1f[:]
                    else:
                        wl = [eW0[:, 3 * l, :], eW0[:, 3 * l + 2, :], eW0[:, 3 * l + 1, :]]
                        bl = eb[:, 3 * l : 3 * l + 1]
                    a2 = mlp_head(
                        [hdst[:], efm[:], hsrc[:, 0, :]], wl,
                        bl, eWs0[:, l, :], eb[:, 3 * l + 1 : 3 * l + 2],
                    )
                    ps3 = p3.tile([128, 4, 128], f32, tag="p3")
                    for j in range(4):
                        nc.tensor.matmul(ps3[:, j, :], a2[:, j * 128 : (j + 1) * 128],
                                         eWs1[:, l, :], start=True, stop=True)
                    rs, nmr = rowln_factors(ps3, 4)
                    tmpf = wp.tile([128, 4, 128], f32, tag="tmpf")
                    for j in range(4):
                        if j % 2 == 0:
                            nc.scalar.activation(tmpf[:, j, :], ps3[:, j, :], AF.Identity,
                                                 bias=nmr[:, j : j + 1], scale=rs[:, j : j + 1])
                        else:
                            nc.vector.tensor_scalar(tmpf[:, j, :], ps3[:, j, :],
                                                    rs[:, j : j + 1], nmr[:, j : j + 1],
                                                    ALU.mult, ALU.add)
                    if fl["eln"]:
                        for j in range(4):
                            nc.vector.tensor_tensor(tmpf[:, j, :], tmpf[:, j, :], elnw[:, l, :], ALU.mult)
                            nc.vector.tensor_tensor(tmpf[:, j, :], tmpf[:, j, :], elnb[:, l, :], ALU.add)
                    if lazy and l == 0:
                        nc.vector.tensor_scalar(e_rm[:, 4 * c : 4 * c + 4, :],
                                                e_rm[:, 4 * c : 4 * c + 4, :],
                                                re[:], nmre[:], ALU.mult, ALU.add)
                    nc.vector.tensor_tensor(e_rm[:, 4 * c : 4 * c + 4, :],
                                            e_rm[:, 4 * c : 4 * c + 4, :], tmpf[:], ALU.add)
                    tmpb = wp.tile([128, 4, 128], bf16, tag="tmpb")
                    nc.scalar.activation(tmpb[:], tmpf[:], AF.Copy)
                    # scatter into window-chained psum (FM out: tmpb is lhsT);
                    # the mean 1/cnt is pre-scaled into seg on the host
                    for j in range(4):
                        t = 4 * c + j
                        wdx = wsched[t]
                        if wfirst[t]:
                            psW = pS.tile([128, 128], f32, tag="pS")
                        nc.tensor.matmul(psW[:], tmpb[:, j, :], segld[:, 0, j, :],
                                         start=wfirst[t], stop=wlast[t])
                        if wlast[t]:
                            nc.vector.tensor_copy(agg_b[:, wdx, :], psW[:])

                for cn in range((NT + 3) // 4):
                    node_chunk(cn)

                if l + 1 < int(os.environ.get("KLAYERS", L)):
                    nc.gpsimd.collective_compute(
                        "AllGather", ALU.bypass, replica_groups=RG,
                        ins=[hsh_d[:]], outs=[htab_d[:]]
                    )

            # ================= DECODER =================
            for c in range(NCH):
                c0 = c * 512
                nt4 = min(4, NT - 4 * c)
                w = nt4 * 128
                a3 = mlp_head(
                    [h_fm[:, 4 * c : 4 * c + nt4, :]], [decW[:, 0, :]],
                    decb[:, 0:1], decW[:, 1, :], decb[:, 1:2], decW[:, 2, :], decb[:, 2:3],
                    width=w,
                )
                psd = p3.tile([128, 4, 128], f32, tag="p3")
                for j in range(nt4):
                    nc.tensor.matmul(psd[:, j, 0:3], a3[:, j * 128 : (j + 1) * 128],
                                     decWl[:], start=True, stop=True)
                ot = wp.tile([128, 4, 3], f32, tag="ot")
                nc.scalar.activation(ot[:, :nt4, :], psd[:, :nt4, 0:3], AF.Copy)
                if fl["decbl"]:
                    for j in range(nt4):
                        nc.vector.tensor_tensor(ot[:, j, :], ot[:, j, :], decbl[:], ALU.add)
                nc.sync.dma_start(
                    out_d[c0 : c0 + w].rearrange("(t p) f -> p t f", p=128),
                    ot[:, :nt4, :])

    nc.compile()
    return nc


def _prep(inputs, cfg):
    """Host-side sharding/index prep. Returns in_maps list."""
    N, E, L = cfg["N"], cfg["E"], cfg["L"]
    NPC, NPAD, ECP = cfg["NPC"], cfg["NPAD"], cfg["EC_PAD"]
    wsched = cfg["wsched"]
    ET = ECP // 128
    NW = NPAD // 128
    f = lambda k: np.asarray(inputs[k], np.float32)
    b = lambda a: np.ascontiguousarray(a).astype(ml_dtypes.bfloat16)

    ei = np.asarray(inputs["edge_index"])
    src_g, dst_g = ei[0].astype(np.int64), ei[1].astype(np.int64)
    ea = f("edge_attr")
    x = f("x")
    cnt = np.bincount(dst_g, minlength=N).astype(np.float32)
    icnt_full = 1.0 / np.maximum(cnt, 1.0)

    tblrow = lambda g: (g // NPC) * NPAD + (g % NPC)

    order = np.argsort(dst_g, kind="stable")
    in_maps = []
    shared = None
    for c in range(NC):
        lo, hi = c * NPC, (c + 1) * NPC
        sel = order[(dst_g[order] >= lo) & (dst_g[order] < hi)]
        dl = dst_g[sel] - lo           # local dst, ascending
        win = dl // 128
        srcv = np.zeros(ECP, np.int64)
        eav = np.zeros((ECP, 3), np.float32)
        msk = np.zeros(ECP, np.float32)
        seg = np.zeros((ET, 128, 128), np.float32)
        seg1 = np.zeros((ET, 128, 128), np.float32)
        pos = {}
        for t, wd in enumerate(wsched):
            pos.setdefault(wd, []).append(t)
        for wd in range(NW):
            idxs = np.where(win == wd)[0]
            tiles = pos.get(wd, [])
            assert len(idxs) <= len(tiles) * 128, (c, wd, len(idxs), len(tiles))
            for k, i in enumerate(idxs):
                t = tiles[k // 128]
                r = k % 128
                g = t * 128 + r
                e_ = sel[i]
                srcv[g] = src_g[e_]
                eav[g] = ea[e_]
                msk[g] = 1.0
                seg[t, r, dl[i] - 128 * wd] = icnt_full[dst_g[e_]]
                seg1[t, r, dl[i] - 128 * wd] = 1.0
        segp = np.zeros((2, ET, 128, 128), np.float32)
        segp[0] = seg
        segp[1] = seg1.transpose(0, 2, 1)
        emask = msk.reshape(ET, 128).T.copy()  # [128, ET]
        nmask = np.zeros((128, NW), np.float32)
        icnt_c = np.ones((128, NW), np.float32)
        for t in range(NW):
            for p in range(128):
                r = t * 128 + p
                if r < NPC:
                    nmask[p, t] = 1.0
                    icnt_c[p, t] = icnt_full[lo + r]
        xT = np.zeros((5, NPAD), np.float32)
        xT[:, :NPC] = x[lo:hi].T
        eaT = eav.T.copy()

        m = {
            "xT": b(xT), "eaT": b(eaT),
            "srci": _wrap_idx(tblrow(srcv).astype(np.int16)),
            "segp": b(segp), "icnt": icnt_c, "emask": emask, "nmask": nmask,
        }
        if shared is None:
            shared = {
                "ident": b(np.eye(128)),
                "ones1": np.ones((1, 128), np.float32),
                "onesK": np.ones((128, 1), np.float32),
                "encNW0": b(f("encN_W0")), "encNW": b(f("encN_Ws")),
                "encEW0": b(f("encE_W0")), "encEW": b(f("encE_Ws")),
                "eW0": b(f("eW0").reshape(L, 3, 128, 128)),
                "eWs0": b(f("eWs")[:, 0]), "eWs1": b(f("eWs")[:, 1]),
                "nW0": b(f("nW0").reshape(L, 2, 128, 128)),
                "nWs0": b(f("nWs")[:, 0]), "nWs1": b(f("nWs")[:, 1]),
                "decW": b(np.stack([f("dec_W0"), f("dec_Ws")[0], f("dec_Ws")[1]])),
                "decWl": b(f("dec_Wl")),
                "encNb": f("encN_bs").T.copy(), "encEb": f("encE_bs").T.copy(),
                "eb": f("ebs").reshape(L * 3, 128).T.copy(),
                "nb": f("nbs").reshape(L * 3, 128).T.copy(),
                "decb": f("dec_bs").T.copy(),
            }
            flg = cfg["flags"]
            if flg["eln"]:
                shared["elnw"] = np.tile(f("elnw")[:, None, :], (1, 128, 1))
                shared["elnb"] = np.tile(f("elnb")[:, None, :], (1, 128, 1))
            if flg["nln"]:
                shared["nlnw"] = np.tile(f("nlnw")[:, None, :], (1, 128, 1))
                shared["nlnb"] = np.tile(f("nlnb")[:, None, :], (1, 128, 1))
            if flg["gln"]:
                shared["gNw"] = np.tile(f("encN_lnw")[None, :], (128, 1))
                shared["gNb"] = np.tile(f("encN_lnb")[None, :], (128, 1))
                shared["gEw"] = np.tile(f("encE_lnw")[None, :], (128, 1))
                shared["gEb"] = np.tile(f("encE_lnb")[None, :], (128, 1))
            if flg["decbl"]:
                shared["decbl"] = np.tile(f("dec_bl")[None, :], (128, 1))
        m.update(shared)
        in_maps.append(m)
    return in_maps


def make_cfg(inputs):
    N = np.asarray(inputs["x"]).shape[0]
    E = np.asarray(inputs["edge_index"]).shape[1]
    L = np.asarray(inputs["eW0"]).shape[0]
    NPC = N // NC
    NPAD = ((NPC + 127) // 128) * 128
    NW = NPAD // 128
    ei = np.asarray(inputs["edge_index"])
    dst = ei[1].astype(np.int64)
    tw = []
    for wd in range(NW):
        mx = 1
        for c in range(NC):
            lo = c * NPC
            nwin = int(((dst >= lo + wd * 128) & (dst < min(lo + (wd + 1) * 128, lo + NPC))).sum())
            mx = max(mx, (nwin + 127) // 128)
        tw.append(mx)
    wsched = []
    for wd in range(NW):
        wsched += [wd] * tw[wd]
    while (len(wsched) * 128) % 512:
        wsched.append(NW - 1)
    flags = {
        "eln": bool(np.any(np.asarray(inputs["elnw"]) != 1) or np.any(np.asarray(inputs["elnb"]) != 0)),
        "nln": bool(np.any(np.asarray(inputs["nlnw"]) != 1) or np.any(np.asarray(inputs["nlnb"]) != 0)),
        "gln": bool(
            np.any(np.asarray(inputs["encN_lnw"]) != 1) or np.any(np.asarray(inputs["encN_lnb"]) != 0)
            or np.any(np.asarray(inputs["encE_lnw"]) != 1) or np.any(np.asarray(inputs["encE_lnb"]) != 0)
        ),
        "decbl": bool(np.any(np.asarray(inputs["dec_bl"]) != 0)),
    }
    return {
        "N": N, "E": E, "L": L, "NPC": NPC, "NPAD": NPAD,
        "EC_PAD": len(wsched) * 128, "wsched": wsched, "flags": flags,
    }


_CACHE = {}


def kernel(**inputs) -> np.ndarray:
    cfg = make_cfg(inputs)
    key = (cfg["N"], cfg["E"], cfg["L"], cfg["EC_PAD"], tuple(sorted(cfg["flags"].items())))
    if key not in _CACHE:
        _CACHE[key] = build(cfg)
    nc = _CACHE[key]
    in_maps = _prep(inputs, cfg)
    res = run_bass_kernel_spmd(nc, in_maps, list(range(NC))).results
    NPC = cfg["NPC"]
    out = np.concatenate([res[c]["out"][:NPC] for c in range(NC)], axis=0)
    return out.astype(np.float32)
